# revision 59
# baseline (speedup 1.0000x reference)
"""MHGCN kernel for 8 Trainium2 NeuronCores (v2).

Row-shard A_stack [7,4096,4096] across 8 cores (512 rows, ~29MB bf16/core).
Per 128-row x 2048-col chunk, engines split the work:
  PE    : merged = sum_r w_r*R_r via scaled-identity PSUM accumulation,
          plus s*tanh accumulated into the same PSUM -> lt = final_A local part.
  GPSIMD: S_j = (R_j > 0) masks and two of the three E-precursor scales.
  DVE   : fused STT/TS chain for arg = sum_j (1.5R_j+S_j) * (sum 0.4 M R).
  ACT   : merged cast (PSUM->SBUF), tanh, lt cast.
  DMA   : A loads, sendbuf stores, and FT built by XBAR-transpose DMA.
The U1|V GEMM accumulates into PSUM *during* phase 1 as FT tiles appear.
The merged-transpose exchange is 4 row-tile AllToAlls (1MB each) overlapped
with phase 1.  All biases are folded into matmuls via an appended ones-row.
struct branch (rank-7) overlaps the V AllGather.  Outputs are l2-normalized
(scale-invariant, so /2 factors are dropped).
"""
import sys

sys.path.insert(0, "/opt/trn_rl_repo")

import numpy as np

import bass_rust
import concourse.bass as bass
import concourse.tile as tile
from concourse import mybir
from concourse.bass_utils import run_bass_kernel_spmd
from concourse.masks import make_identity
from concourse.vector_clock import ScopedClock

F32 = mybir.dt.float32
BF16 = mybir.dt.bfloat16
AF = mybir.ActivationFunctionType
OP = mybir.AluOpType

P = 128
N = 4096
NFEAT = 128
OUT = 64
NREL = 7
NCORES = 8
ROWS = N // NCORES        # 512 rows per core
NT = ROWS // P            # 4 row tiles per core
KT = N // P               # 32 global k tiles
C = 1024                  # streaming column chunk
NCH = N // C              # 4 chunks per row tile
TPC = C // P              # 8 k tiles per chunk
NDB = C // ROWS if C >= ROWS else 0  # dest blocks per chunk (C=1024 -> 2)

GPS_OFFLOAD = False       # gpsimd elementwise measured ~26x slower than DVE


def _patched_drain_and_barrier(self, tick_clock, wait_clock):
    # Stock Tile attaches every outstanding proc's sem wait to one Drain;
    # this walrus build caps sync waits per instruction, so split them
    # into single-wait drains.
    drain_inst = self.nc.sync.drain()
    wait_clock.add_sem_waits(
        drain_inst.ins, ScopedClock({None: tick_clock.global_clock})
    )
    si = drain_inst.ins.sync_info
    if si is not None and len(si.on_wait) > 1:
        waits = list(si.on_wait)
        si.on_wait = [waits[0]]
        for w in waits[1:]:
            extra = self.nc.sync.drain()
            extra.ins.sync_info = bass_rust.SyncInfo(on_wait=[w], on_update=[])
    self.nc.all_engine_barrier()
    assert self.sems is not None
    popped = self.nc._tile_sem_poison_stack.pop()
    assert popped is self._sem_poison
    self.nc.clear_and_free_semaphores(list(self.sems.allocated().values()))
    self.nc.all_engine_barrier()


tile.TileContext._drain_and_barrier = _patched_drain_and_barrier


def _split_multi_waits(nc, limit=1):
    """Walrus in this container caps sync-wait commands per instruction.
    Hoist all-but-`limit` waits of any instruction onto single-wait NoOps
    inserted just before it on the same engine queue."""
    cnt = 0
    for fn in nc.m.functions:
        for blk in fn.blocks:
            lst = list(blk.instructions)
            out = []
            changed = False
            for inst in lst:
                si = inst.sync_info
                if si is not None and len(si.on_wait) > limit:
                    waits = list(si.on_wait)
                    for w in waits[:-limit]:
                        n = bass_rust.InstNoOp(name=f"wsplit-{cnt}")
                        cnt += 1
                        n.engine = inst.engine
                        n.bass_nofuse = True
                        n.sync_info = bass_rust.SyncInfo(on_wait=[w],
                                                         on_update=[])
                        nc.register_instruction(n, overwrite=True)
                        out.append(n)
                    si.on_wait = waits[-limit:]
                    changed = True
                out.append(inst)
            if changed:
                blk.instructions = out
    return cnt


def build_nc(cc, s):
    """cc: 3x3 tuple of floats = 0.4 * M_ij (zero diag); s: interaction str."""
    nc = bass.Bass()

    a4 = nc.dram_tensor("a4", [NT, P, NREL, N], BF16, kind="ExternalInput")
    featT = nc.dram_tensor("featT", [NFEAT, N], BF16, kind="ExternalInput")
    W1G = nc.dram_tensor("W1G", [NFEAT, 2 * OUT], BF16, kind="ExternalInput")
    W2b = nc.dram_tensor("W2b", [OUT, OUT], BF16, kind="ExternalInput")
    biasrow = nc.dram_tensor("biasrow", [1, NT * 2 * OUT], BF16,
                             kind="ExternalInput")
    b2row4 = nc.dram_tensor("b2row4", [1, NT * OUT], BF16, kind="ExternalInput")
    b1row = nc.dram_tensor("b1row", [1, OUT], BF16, kind="ExternalInput")
    b2row = nc.dram_tensor("b2row", [1, OUT], BF16, kind="ExternalInput")
    wI = nc.dram_tensor("wI", [8, P, P], BF16, kind="ExternalInput")
    encb = nc.dram_tensor("encb", [P, KT * NREL], BF16, kind="ExternalInput")
    encT8 = nc.dram_tensor("encT8", [8, N], BF16, kind="ExternalInput")
    encRT8 = nc.dram_tensor("encRT8", [8, ROWS], BF16, kind="ExternalInput")
    swt = nc.dram_tensor("swt", [NREL, 1], F32, kind="ExternalInput")

    o_res = nc.dram_tensor("o_res", [ROWS, OUT], F32, kind="ExternalOutput")
    o_b1 = nc.dram_tensor("o_b1", [ROWS, OUT], F32, kind="ExternalOutput")
    o_b2 = nc.dram_tensor("o_b2", [ROWS, OUT], F32, kind="ExternalOutput")

    groups = [list(range(NCORES))]

    with tile.TileContext(nc) as tc:
        with (
            tc.tile_pool(name="persist", bufs=1) as pp,
            tc.tile_pool(name="dram", bufs=1, space="DRAM") as dpool,
            tc.tile_pool(name="uvpsum", bufs=1, space="PSUM") as uvp,
        ):
            # ---- small persistent tensors ----
            ident = pp.tile([P, P], F32)
            make_identity(nc, ident)
            identb = pp.tile([P, P], BF16)
            nc.vector.tensor_copy(identb[:], ident[:])
            onesrow = pp.tile([1, P], BF16)
            nc.vector.memset(onesrow[:], 1.0)

            wIs = pp.tile([P, 8, P], BF16)
            nc.scalar.dma_start(out=wIs[:], in_=wI[:].rearrange("r p c -> p r c"))
            W1Gs = pp.tile([NFEAT, 2 * OUT], BF16)
            nc.scalar.dma_start(out=W1Gs[:], in_=W1G[:])
            W2s = pp.tile([OUT, OUT], BF16)
            nc.scalar.dma_start(out=W2s[:], in_=W2b[:])
            biasr = pp.tile([1, NT * 2 * OUT], BF16)
            nc.scalar.dma_start(out=biasr[:], in_=biasrow[:])
            b2r4 = pp.tile([1, NT * OUT], BF16)
            nc.scalar.dma_start(out=b2r4[:], in_=b2row4[:])
            b1r = pp.tile([1, OUT], BF16)
            nc.scalar.dma_start(out=b1r[:], in_=b1row[:])
            b2r = pp.tile([1, OUT], BF16)
            nc.scalar.dma_start(out=b2r[:], in_=b2row[:])
            encbs = pp.tile([P, KT * NREL], BF16)
            nc.scalar.dma_start(out=encbs[:], in_=encb[:])
            encT8s = pp.tile([8, N], BF16)
            nc.scalar.dma_start(out=encT8s[:], in_=encT8[:])
            encRT8s = pp.tile([8, ROWS], BF16)
            nc.scalar.dma_start(out=encRT8s[:], in_=encRT8[:])
            swts = pp.tile([NREL, 1], F32)
            nc.scalar.dma_start(out=swts[:], in_=swt[:])

            # ---- big persistent tensors ----
            FT = pp.tile([P, KT, ROWS], BF16)      # final_A^T tiles, 32KB/part
            YG = pp.tile([P, KT * 2 * OUT], BF16)  # [Y1 | G] per k-tile, 8KB

            # ---- DRAM bounce buffers ----
            sendb = [dpool.tile([NCORES, P, ROWS], BF16, tag=f"snd{i}",
                                name=f"sendb{i}") for i in range(NT)]
            recvb = [dpool.tile([NCORES, P, ROWS], BF16, tag=f"rcv{i}",
                                name=f"recvb{i}") for i in range(NT)]
            agin = dpool.tile([NT, P, OUT], BF16)
            agout = dpool.tile([NCORES, NT, P, OUT], BF16, addr_space="Shared")

            fbf = pp.tile([NFEAT, N], BF16)
            psum_uv = uvp.tile([P, NT * 2 * OUT], F32)

            # ---- phase 1: stream A row-block, build FT + U1V local ----
            rcvt = {}
            with (
                tc.tile_pool(name="rstr", bufs=2) as prr,
                tc.tile_pool(name="sstr", bufs=2) as pss,
                tc.tile_pool(name="estr", bufs=1) as pes,
                tc.tile_pool(name="mpsum", bufs=1, space="PSUM") as mps,
            ):
                def emit_prep():
                    # deferred prep (emitted after chunk 0 so the stream
                    # starts immediately): YG = [feat@W1 | feat@W1@W2],
                    # U1|V psum bias-init
                    nc.sync.dma_start(out=fbf[:], in_=featT[:])
                    for kt in range(KT):
                        pmy = mps.tile([P, 2 * OUT], F32, tag="yg", bufs=1,
                                       name="pmy")
                        nc.tensor.matmul(pmy[:],
                                         lhsT=fbf[:, kt * P:(kt + 1) * P],
                                         rhs=W1Gs[:], start=True, stop=True)
                        dst = YG[:, kt * 2 * OUT:(kt + 1) * 2 * OUT]
                        if kt % 2:
                            nc.scalar.activation(dst, pmy[:], AF.Copy)
                        else:
                            nc.vector.tensor_copy(dst, pmy[:])
                    nc.tensor.matmul(psum_uv[:], lhsT=onesrow[:], rhs=biasr[:],
                                     start=True, stop=False)
                def emit_uv_local(ch):
                    pi, pq = ch
                    for t in range(TPC):
                        kt = pq * TPC + t
                        nc.tensor.matmul(
                            psum_uv[:, pi * P:(pi + 1) * P],
                            lhsT=FT[:, kt, pi * P:(pi + 1) * P],
                            rhs=YG[:, kt * 2 * OUT:(kt + 1) * 2 * OUT],
                            start=False, stop=False)

                def emit_uv_recv(j, stop_last=False):
                    rt = rcvt[j]
                    for d in range(NCORES):
                        kt = 4 * d + j
                        for ib in range(NT):
                            last = (stop_last and d == NCORES - 1
                                    and ib == NT - 1)
                            nc.tensor.matmul(
                                psum_uv[:, ib * P:(ib + 1) * P],
                                lhsT=rt[:, d, ib * P:(ib + 1) * P],
                                rhs=YG[:, kt * 2 * OUT:(kt + 1) * 2 * OUT],
                                start=False, stop=last)

                def emit_lt(p):
                    # deferred: lt = s*tanh + mrow, then PE-transpose into FT
                    # (XBAR-transpose DMA measured to serialize against the
                    # AllToAlls; bf16 psum ACCUMULATION is broken on HW)
                    th, mrow, pi, pq = p
                    t2 = pss.tile([P, C], BF16, tag="t2", bufs=1)
                    nc.vector.tensor_scalar(t2[:], th[:], s, None, OP.mult)
                    lt = pss.tile([P, C], BF16, tag="lt", bufs=1)
                    nc.vector.tensor_tensor(lt[:], t2[:], mrow[:], OP.add)
                    trp = mps.tile([P, TPC * P], BF16, tag="tr", bufs=1,
                                   name="trp")
                    for t in range(TPC):
                        nc.tensor.transpose(trp[:, t * P:(t + 1) * P],
                                            lt[:, t * P:(t + 1) * P],
                                            identb[:])
                    nc.scalar.activation(
                        FT[:, pq * TPC:(pq + 1) * TPC, pi * P:(pi + 1) * P],
                        trp[:].rearrange("p (t m) -> p t m", t=TPC), AF.Copy)

                NG = NT * NCH
                pairs = [(j, o) for j in range(3) for o in range(3) if o != j]

                def emit_loads(g):
                    i, q = divmod(g, NCH)
                    c0 = q * C
                    r3 = prr.tile([P, 3, C], BF16, tag="r3", name="r3", bufs=4)
                    nc.sync.dma_start(out=r3[:], in_=a4[i, :, 0:3, c0:c0 + C])
                    r4 = prr.tile([P, 4, C], BF16, tag="r4", name="r4", bufs=4)
                    nc.sync.dma_start(out=r4[:], in_=a4[i, :, 3:7, c0:c0 + C])
                    return (r3, r4)

                def emit_scales(rr):
                    # four of six E-precursor scale ops on ACT (the other
                    # two run on DVE inside the main body)
                    r3, r4 = rr
                    sc = {}
                    for n, (j, o) in enumerate(pairs[:4]):
                        t_ = pes.tile([P, C], BF16, tag=f"sc{n}", bufs=2,
                                      name=f"sc{n}")
                        nc.scalar.mul(t_[:], r3[:, o, :], cc[j][o])
                        sc[(j, o)] = t_
                    return sc

                pend_lt = None
                pend_uv = []
                rr = emit_loads(0)
                rr_next = emit_loads(1)
                rr_next2 = emit_loads(2)
                sc = emit_scales(rr)
                for g in range(NG):
                    i, q = divmod(g, NCH)
                    r3, r4 = rr
                    R = lambda j: r3[:, j, :] if j < 3 else r4[:, j - 3, :]
                    if g + 3 < NG:
                        rr_next3 = emit_loads(g + 3)

                    # PE: merged accumulation (w_r * I) @ R_r
                    pm = mps.tile([P, C], F32, tag="m", name="pm", bufs=2)
                    for t in range(C // 512):
                        sl = slice(t * 512, (t + 1) * 512)
                        for r in range(NREL):
                            nc.tensor.matmul(
                                pm[:, sl], lhsT=wIs[:, r, :], rhs=R(r)[:, sl],
                                start=(r == 0), stop=(r == NREL - 1))

                    # ACT: merged cast psum -> sbuf bf16; scatter to A2A buf
                    mrow = pss.tile([P, C], BF16, tag="mrow", bufs=2)
                    nc.scalar.activation(mrow[:], pm[:], AF.Copy)
                    nc.scalar.dma_start(
                        out=sendb[i][NDB * q:NDB * q + NDB, :, :].rearrange(
                            "j p c -> p j c"),
                        in_=mrow[:].rearrange("p (j c) -> p j c", j=NDB))

                    # ACT: E-precursor scales for the NEXT chunk (so they
                    # never sit behind this chunk's tanh in the ACT queue)
                    if g + 1 < NG:
                        sc_next = emit_scales(rr_next)

                    # arg = 1.5 * sum_j (R_j + (2/3) S_j) * E_j, with
                    # E_j = sum_{o!=j} 0.4 M_jo R_o ; S_j = (R_j > 0).
                    # The 1.5 is folded into the tanh input scale.
                    # DVE: S' = (R>0)*(2/3) (dual-op TS), sixth scale, Q'
                    S = []
                    for j in range(3):
                        sj = pss.tile([P, C], BF16, tag=f"s{j}", bufs=1,
                                      name=f"sj{j}")
                        nc.vector.tensor_scalar(sj[:], R(j), 0.0, 2.0 / 3.0,
                                                OP.is_gt, OP.mult)
                        S.append(sj)
                    for n in (4, 5):
                        jn, on = pairs[n]
                        tn = pes.tile([P, C], BF16, tag=f"sc{n}", bufs=1,
                                      name=f"sc{n}")
                        nc.vector.tensor_scalar(tn[:], R(on), cc[jn][on], None,
                                                OP.mult)
                        sc[(jn, on)] = tn
                    for j in range(3):
                        nc.vector.tensor_tensor(S[j][:], R(j), S[j][:], OP.add)

                    # DVE: E_j (in-place into first scale operand)
                    E = []
                    for j in range(3):
                        o1, o2 = [x for x in range(3) if x != j]
                        ej = sc[(j, o1)]
                        nc.vector.tensor_tensor(ej[:], ej[:],
                                                sc[(j, o2)][:], OP.add)
                        E.append(ej)

                    # DVE: deferred lt of the previous chunk, placed late so
                    # the previous tanh has certainly finished
                    if pend_lt is not None:
                        emit_lt(pend_lt)
                        pend_lt = None

                    # DVE: P = Q'*E
                    for j in range(3):
                        nc.vector.tensor_tensor(E[j][:], S[j][:], E[j][:],
                                                OP.mult)

                    # PE: U1V local accumulation, two chunks behind
                    pend_uv.append((i, q))
                    if len(pend_uv) > 2:
                        emit_uv_local(pend_uv.pop(0))

                    # PE: arg = sum_j P_j into psum (reuse merged banks)
                    pm2 = mps.tile([P, C], F32, tag="m", name="pm2", bufs=2)
                    for t in range(C // 512):
                        sl = slice(t * 512, (t + 1) * 512)
                        for j in range(3):
                            nc.tensor.matmul(
                                pm2[:, sl], lhsT=identb[:], rhs=E[j][:, sl],
                                start=(j == 0), stop=(j == 2))
                    # ACT: tanh(1.5 * arg)
                    th = pss.tile([P, C], BF16, tag="tanh", bufs=2)
                    nc.scalar.activation(th[:], pm2[:], AF.Tanh, scale=1.5)
                    pend_lt = (th, mrow, i, q)
                    if g == 0:
                        emit_prep()
                    if g + 1 < NG:
                        rr, sc = rr_next, sc_next
                    if g + 2 < NG:
                        rr_next = rr_next2
                    if g + 3 < NG:
                        rr_next2 = rr_next3

                    if q == NCH - 1:
                        # row-tile done: fire its AllToAll
                        nc.gpsimd.collective_compute(
                            "AllToAll", OP.bypass, replica_groups=groups,
                            ins=[sendb[i][:].opt()], outs=[recvb[i][:].opt()])

                # drain the pipeline: last lt/xbar, last two uv chunks
                emit_lt(pend_lt)
                for ch in pend_uv:
                    emit_uv_local(ch)

            # ---- phase 2 ----
            with (
                tc.tile_pool(name="rcv", bufs=1) as rcvpool,
                tc.tile_pool(name="post", bufs=1) as post,
                tc.tile_pool(name="postpsum", bufs=1, space="PSUM") as pops,
                tc.tile_pool(name="u2psum", bufs=1, space="PSUM") as u2p,
            ):
                # fetch the exchanged merged columns (j<3 are long done; the
                # A2A#3 tail overlaps the j<3 processing below).  rt3 is
                # loaded per-block so its U1V matmuls pipeline with the DMA.
                for j in range(NT):
                    rt = rcvpool.tile([P, NCORES, ROWS], BF16, tag=f"rt{j}",
                                      bufs=1, name=f"rcvt{j}")
                    if j < 3:
                        eng = nc.sync if j % 2 == 0 else nc.scalar
                        eng.dma_start(
                            out=rt[:],
                            in_=recvb[j][:].rearrange("d p m -> p d m"))
                    else:
                        for dd in range(NCORES):
                            eng = nc.sync if dd % 2 == 0 else nc.scalar
                            eng.dma_start(out=rt[:, dd, :],
                                          in_=recvb[j][dd, :, :])
                    rcvt[j] = rt

                # FT += recv (for U2), recv U1V matmuls
                for j in (0, 1, 2):
                    for d in range(NCORES):
                        kt = 4 * d + j
                        nc.vector.tensor_tensor(FT[:, kt, :], FT[:, kt, :],
                                                rcvt[j][:, d, :], OP.add)
                for j in (0, 1, 2):
                    emit_uv_recv(j)
                emit_uv_recv(3, stop_last=True)
                for d in range(NCORES):
                    kt = 4 * d + 3
                    nc.vector.tensor_tensor(FT[:, kt, :], FT[:, kt, :],
                                            rcvt[3][:, d, :], OP.add)

                # U1 rows (f32) and V rows (bf16) out of psum
                u1sb = post.tile([P, NT * OUT], F32)
                vsb = post.tile([P, NT, OUT], BF16)
                for ib in range(NT):
                    nc.scalar.activation(u1sb[:, ib * OUT:(ib + 1) * OUT],
                                         psum_uv[:, ib * P:ib * P + OUT],
                                         AF.Copy)
                    nc.scalar.activation(vsb[:, ib, :],
                                         psum_uv[:, ib * P + OUT:(ib + 1) * P],
                                         AF.Copy)
                nc.sync.dma_start(out=agin[:].rearrange("i p f -> p i f"),
                                  in_=vsb[:])
                nc.gpsimd.collective_compute(
                    "AllGather", OP.bypass, replica_groups=groups,
                    ins=[agin[:].opt()], outs=[agout[:].opt()])

                # ---- struct branch (overlaps AllGather) ----
                ph1 = pops.tile([NREL, OUT], F32, tag="pp7")
                for kt in range(KT):
                    nc.tensor.matmul(
                        ph1[:], lhsT=encbs[:, kt * NREL:(kt + 1) * NREL],
                        rhs=YG[:, kt * 2 * OUT:kt * 2 * OUT + OUT],
                        start=(kt == 0), stop=(kt == KT - 1))
                H1p8 = post.tile([8, OUT], BF16)
                nc.scalar.activation(H1p8[0:NREL, :], ph1[:], AF.Copy,
                                     scale=swts[:])
                nc.sync.dma_start(out=H1p8[NREL:8, :], in_=b1r[:])

                U3sb = post.tile([P, KT * OUT], BF16)
                for kt in range(KT):
                    pm3 = pops.tile([P, OUT], F32, tag="ppu3", bufs=1)
                    nc.tensor.matmul(pm3[:],
                                     lhsT=encT8s[:, kt * P:(kt + 1) * P],
                                     rhs=H1p8[:], start=True, stop=True)
                    dst = U3sb[:, kt * OUT:(kt + 1) * OUT]
                    if kt % 2:
                        nc.scalar.activation(dst, pm3[:], AF.Copy)
                    else:
                        nc.vector.tensor_copy(dst, pm3[:])

                ph2 = pops.tile([NREL, OUT], F32, tag="pp7")
                for kt in range(KT):
                    nc.tensor.matmul(
                        ph2[:], lhsT=encbs[:, kt * NREL:(kt + 1) * NREL],
                        rhs=U3sb[:, kt * OUT:(kt + 1) * OUT],
                        start=(kt == 0), stop=(kt == KT - 1))
                H2s = post.tile([NREL, OUT], BF16)
                nc.scalar.activation(H2s[:], ph2[:], AF.Copy, scale=swts[:])
                ptr = pops.tile([OUT, NREL], BF16, tag="pptr")
                nc.tensor.transpose(ptr[:], H2s[:], identb[:NREL, :NREL])
                H2T = post.tile([OUT, NREL], BF16)
                nc.vector.tensor_copy(H2T[:], ptr[:])
                pg2 = pops.tile([NREL, OUT], F32, tag="pp7")
                nc.tensor.matmul(pg2[:], lhsT=H2T[:], rhs=W2s[:],
                                 start=True, stop=True)
                G2p8 = post.tile([8, OUT], BF16)
                nc.scalar.activation(G2p8[0:NREL, :], pg2[:], AF.Copy)
                nc.sync.dma_start(out=G2p8[NREL:8, :], in_=b2r[:])

                u4sb = post.tile([P, NT * OUT], F32)
                for ib in range(NT):
                    pm4 = pops.tile([P, OUT], F32, tag="ppu4", bufs=1)
                    nc.tensor.matmul(pm4[:],
                                     lhsT=encRT8s[:, ib * P:(ib + 1) * P],
                                     rhs=G2p8[:], start=True, stop=True)
                    nc.scalar.activation(u4sb[:, ib * OUT:(ib + 1) * OUT],
                                         pm4[:], AF.Copy)

                # ---- layer 2 (transposed): U2^T = Y2^T @ final_A^T + b2
                # One wide matmul per k-tile instead of 4 narrow ones.
                y2sb = post.tile([P, KT, OUT], BF16)
                nc.sync.dma_start(
                    out=y2sb[:],
                    in_=agout[:].rearrange("d i p f -> p (d i) f"))
                ones512 = post.tile([1, ROWS], BF16)
                nc.vector.memset(ones512[:], 1.0)
                psum_u2T = u2p.tile([OUT, ROWS], F32)
                nc.tensor.matmul(psum_u2T[:], lhsT=b2r[:], rhs=ones512[:],
                                 start=True, stop=False)
                for kt in range(KT):
                    nc.tensor.matmul(psum_u2T[:], lhsT=y2sb[:, kt, :],
                                     rhs=FT[:, kt, :],
                                     start=False, stop=(kt == KT - 1))
                u2tc = post.tile([OUT, ROWS], F32)
                nc.vector.tensor_copy(u2tc[:], psum_u2T[:])
                u2sb = post.tile([P, NT * OUT], F32)
                for ib in range(NT):
                    ptu = pops.tile([P, OUT], F32, tag="ppu2", bufs=2)
                    nc.tensor.transpose(ptu[:], u2tc[:, ib * P:(ib + 1) * P],
                                        ident[:OUT, :OUT])
                    if ib % 2:
                        nc.scalar.activation(
                            u2sb[:, ib * OUT:(ib + 1) * OUT], ptu[:], AF.Copy)
                    else:
                        nc.vector.tensor_copy(
                            u2sb[:, ib * OUT:(ib + 1) * OUT], ptu[:])

                # ---- combine + normalize + store ----
                xsb = post.tile([P, NT * OUT], F32)      # U1+U2 (branch1)
                nc.vector.tensor_tensor(xsb[:], u1sb[:], u2sb[:], OP.add)
                rsb = post.tile([P, NT * OUT], F32)      # U1+U2+2*U4 (result)
                nc.vector.scalar_tensor_tensor(rsb[:], u4sb[:], 2.0, xsb[:],
                                               OP.mult, OP.add)
                for vec, od in ((rsb, o_res), (xsb, o_b1), (u4sb, o_b2)):
                    sq = post.tile([P, NT * OUT], F32, tag="nsq", bufs=2)
                    nc.vector.tensor_tensor(sq[:], vec[:], vec[:], OP.mult)
                    ss = post.tile([P, NT], F32, tag="nss", bufs=2)
                    nc.vector.tensor_reduce(
                        ss[:], sq[:].rearrange("p (b f) -> p b f", b=NT),
                        mybir.AxisListType.X, OP.add)
                    nr = post.tile([P, NT], F32, tag="nnr", bufs=2)
                    nc.scalar.activation(nr[:], ss[:], AF.Sqrt)
                    nc.vector.tensor_scalar(nr[:], nr[:], 1e-12, None, OP.max)
                    ninv = post.tile([P, NT], F32, tag="ninv", bufs=2)
                    nc.vector.reciprocal(ninv[:], nr[:])
                    osb = post.tile([P, NT, OUT], F32, tag="nosb", bufs=2)
                    for ib in range(NT):
                        nc.vector.tensor_scalar(
                            osb[:, ib, :], vec[:, ib * OUT:(ib + 1) * OUT],
                            ninv[:, ib:ib + 1], None, OP.mult)
                    nc.sync.dma_start(
                        out=od[:].rearrange("(i p) f -> p i f", p=P),
                        in_=osb[:])

    _split_multi_waits(nc)
    return nc


_NC_CACHE = {}


def get_nc(cc, s):
    key = (tuple(tuple(row) for row in cc), s)
    if key not in _NC_CACHE:
        _NC_CACHE[key] = build_nc(cc, s)
    return _NC_CACHE[key]


def make_in_maps(feature, A_stack, encode, W1, b1, W2, b2, weight_b,
                 relation_interaction, interaction_strength, struct_weight):
    import ml_dtypes
    bf = lambda x: np.ascontiguousarray(np.asarray(x, np.float32)
                                        .astype(ml_dtypes.bfloat16))
    f32 = lambda x: np.ascontiguousarray(np.asarray(x, dtype=np.float32))

    W1 = np.asarray(W1, np.float32)
    W2 = np.asarray(W2, np.float32)
    b1 = np.reshape(np.asarray(b1, np.float32), (1, OUT))
    b2 = np.reshape(np.asarray(b2, np.float32), (1, OUT))
    w = np.asarray(weight_b, np.float32)[:, 0]
    s = float(np.asarray(interaction_strength, np.float32).reshape(-1)[0])
    M = np.asarray(relation_interaction, np.float32) * (1.0 - np.eye(3,
                                                        dtype=np.float32))
    cc = tuple(tuple(float(0.4 * M[i, j]) for j in range(3)) for i in range(3))

    wI = np.zeros((8, P, P), np.float32)
    for r in range(NREL):
        wI[r] = w[r] * np.eye(P, dtype=np.float32)
    wI[7] = s * np.eye(P, dtype=np.float32)

    enc = np.asarray(encode, np.float32)
    encb = enc.reshape(KT, P, NREL).transpose(1, 0, 2).reshape(P, KT * NREL)
    encT8 = np.ones((8, N), np.float32)
    encT8[0:NREL] = enc.T

    common = dict(
        featT=bf(np.asarray(feature, np.float32).T),
        W1G=bf(np.concatenate([W1, W1 @ W2], axis=1)),
        W2b=bf(W2),
        biasrow=bf(np.tile(np.concatenate([b1, b1 @ W2], axis=1), (1, NT))),
        b2row4=bf(np.tile(b2, (1, NT))),
        b1row=bf(b1),
        b2row=bf(b2),
        wI=bf(wI),
        encb=bf(encb),
        encT8=bf(encT8),
        swt=f32(np.reshape(struct_weight, (NREL, 1))),
    )
    in_maps = []
    A = np.asarray(A_stack, np.float32).astype(ml_dtypes.bfloat16)
    for c in range(NCORES):
        rows = slice(c * ROWS, (c + 1) * ROWS)
        m = dict(common)
        m["a4"] = np.ascontiguousarray(
            A[:, rows, :].transpose(1, 0, 2).reshape(NT, P, NREL, N))
        encRT8 = np.ones((8, ROWS), np.float32)
        encRT8[0:NREL] = enc[rows].T
        m["encRT8"] = bf(encRT8)
        in_maps.append(m)
    return in_maps, cc, s


def run(inputs, trace=False, tmpdir=None):
    in_maps, cc, s = make_in_maps(**inputs)
    nc = get_nc(cc, s)
    kres = run_bass_kernel_spmd(nc, in_maps, list(range(NCORES)),
                                trace=trace, tmpdir=tmpdir)
    res = kres.results
    result = np.concatenate([res[c]["o_res"] for c in range(NCORES)], axis=0)
    branch1 = np.concatenate([res[c]["o_b1"] for c in range(NCORES)], axis=0)
    branch2 = np.concatenate([res[c]["o_b2"] for c in range(NCORES)], axis=0)
    return (result, branch1, branch2), kres


def kernel(**inputs):
    return run(inputs)[0]
